# revision 22
# baseline (speedup 1.0000x reference)
"""Trainium2 Bass kernel for nn_DualAttention (S=2048, B=16, H2=2048, V=1024).

Computation (per the reference):
    sum_w = hidden @ Ww + bw + z @ Wz + bz + w_a*0.5        [S, B, V]
    u     = tanh(sum_w) @ Vw + vb                            [S, B, 1]
    out   = softmax(u, axis=0)                               [S, B, 1]

Strategy
--------
Data-parallel over batch: 16 batches -> 2 per NeuronCore (8 cores).
Host-side prep per core:
  * concat hidden/z along the hidden axis -> X [ROWS=4096, H=4096]
    (rows are b-major: row = b_local*2048 + s)
  * transpose to xt = X^T [H, ROWS], cast to the matmul dtype
  * W = concat([Ww, Wz], 0) [H, V], reordered into per-(vb,k) 128x128
    tiles; bias = bw + bz + 0.5*w_a
Device kernel (per core), W-stationary matmul with psum layout [v, rows]:
  for each rowblock (RB rows):
    load xt[:, rowblock] into SBUF (one [128, RB] tile per k)
    for vb in 0..7:                       # 128-wide slices of V
      psum[vb] += sum_k W[vb,k].T @ xt[k]      (32 accumulating matmuls)
      t = tanh(psum + bias_vb)            # one ACT op, per-partition bias
      u_psum += Vw[vb].T @ t              # [1, RB] second-stage matmul (f32r)
    u_scratch[rowblock] = u_psum          # via SBUF bounce -> DRAM
  softmax over s per batch (no max subtraction: u is tanh-bounded):
    DMA u_scratch -> [2, 2048], exp+rowsum on ACT (in place),
    reciprocal + scale on DVE (in place), DMA out [2, 2048].

The vb scalar is dropped: softmax is shift-invariant.

MAIN_DT selects the matmul dtype: "bf16" (faster, ~1e-2 rel err) or
"f32r" (fp32 data with the PE's fast rounded-fp32 mode, ~1e-3 rel err).
"""

import numpy as np
import ml_dtypes

# ---------------------------------------------------------------------------
# Problem constants (hardcoded; kernel.py must be self-contained)
# ---------------------------------------------------------------------------
S, B, H2, V = 2048, 16, 2048, 1024
ALPHA_S = 0.5
NCORES = 8
BC = B // NCORES            # local batches per core
ROWS = S * BC               # 4096 rows per core (b-major)
H = 2 * H2                  # 4096 contraction dim (hidden ++ z)
P = 128
NK = H // P                 # 32
NVB = V // P                # 8

MAIN_DT = "bf16"            # "bf16" | "f32r"
RB = 512 if MAIN_DT == "bf16" else 256
NRB = ROWS // RB


# ---------------------------------------------------------------------------
# Workarounds for this walrus build's 1-sync-wait-per-instruction limit
# ---------------------------------------------------------------------------
def _install_drain_patch():
    import concourse.mybir as mybir
    from concourse.tile import TileContext
    from concourse.vector_clock import ScopedClock

    def _drain_and_barrier(self, tick_clock, wait_clock):
        nc = self.nc
        drain_inst = nc.sync.drain()
        wait_clock.add_sem_waits(
            drain_inst.ins, ScopedClock({None: tick_clock.global_clock})
        )
        si = drain_inst.ins.sync_info
        if si is not None:
            waits = list(si.on_wait)
            if len(waits) > 1:
                si.on_wait = [waits[0]]
                for w in waits[1:]:
                    nop = nc.sync.nop(nofuse=True)
                    nop.ins.sync_info = mybir.SyncInfo(on_wait=[w], on_update=[])
        nc.all_engine_barrier()
        assert self.sems is not None
        popped = nc._tile_sem_poison_stack.pop()
        assert popped is self._sem_poison
        nc.clear_and_free_semaphores(list(self.sems.allocated().values()))
        nc.all_engine_barrier()

    TileContext._drain_and_barrier = _drain_and_barrier


def _split_multiwait(nc):
    """Hoist extra sync waits onto same-engine event-semaphore instructions
    inserted just before the carrying instruction."""
    import concourse.mybir as mybir

    counter = 0
    for fn in nc.m.functions:
        for bb in fn.blocks:
            insts = bb.instructions
            new_list = []
            changed = False
            for inst in insts:
                si = inst.sync_info
                if si is not None:
                    waits = list(si.on_wait)
                    if len(waits) > 1:
                        for w in waits[:-1]:
                            counter += 1
                            nop = mybir.InstEventSemaphore(
                                name=f"I-mwsplit-{counter}"
                            )
                            nop.engine = inst.engine
                            nop.bass_nofuse = True
                            nop.sync_info = mybir.SyncInfo(
                                on_wait=[w], on_update=[]
                            )
                            nc.register_instruction(nop)
                            new_list.append(nop)
                        si.on_wait = [waits[-1]]
                        changed = True
                new_list.append(inst)
            if changed:
                bb.instructions = new_list
    return counter


# ---------------------------------------------------------------------------
# Kernel build
# ---------------------------------------------------------------------------
def _build_nc():
    import concourse.bass as bass
    import concourse.mybir as mybir
    from concourse.tile import TileContext

    f32 = mybir.dt.float32
    f32r = mybir.dt.float32r
    bf16 = mybir.dt.bfloat16
    DT = mybir.dt.bfloat16 if MAIN_DT == "bf16" else f32r

    nc = bass.Bass()
    # W pre-tiled host-side: tile (vb, k) is [P, 128] contiguous
    w_d = nc.declare_dram_parameter("w", [NVB, P, NK * P], DT, isOutput=False)
    xt_d = nc.declare_dram_parameter("xt", [H, ROWS], DT, isOutput=False)
    bct_d = nc.declare_dram_parameter("bct", [P, NVB], f32, isOutput=False)
    vwt_d = nc.declare_dram_parameter("vwt", [P, NVB], f32, isOutput=False)
    ones_d = nc.declare_dram_parameter("ones", [P, 1], f32r, isOutput=False)
    att_d = nc.declare_dram_parameter("att", [BC, S], f32, isOutput=True)

    RPB = NRB // BC             # rowblocks per local batch

    with TileContext(nc) as tc:
        with (
            tc.tile_pool(name="wpool", bufs=1) as wpool,
            tc.tile_pool(name="xpool", bufs=1) as xpool,
            tc.tile_pool(name="tpool", bufs=1) as tpool,
            tc.tile_pool(name="spool", bufs=1) as spool,
            tc.tile_pool(name="pspool", bufs=1, space="PSUM") as pspool,
        ):
            # --- resident weights: vb0's tiles first (fast start), then rest
            # each vb's weights may be split into `nsplit` tiles along k so
            # the first matmuls can start before the whole slab lands
            w_sb = [None] * NVB

            def load_w(vb, nsplit=1, issue=True):
                kc = NK // nsplit
                tiles = []
                for j in range(nsplit):
                    t = wpool.tile([P, kc, P], DT, name=f"w_{vb}_{j}")
                    tiles.append(t)
                w_sb[vb] = (tiles, kc)
                if issue:
                    for j in range(nsplit):
                        issue_w(vb, j)

            def issue_w(vb, j):
                tiles, kc = w_sb[vb]
                nc.sync.dma_start(
                    out=tiles[j][:],
                    in_=w_d[vb, :, j * kc * P : (j + 1) * kc * P].rearrange(
                        "p (k q) -> p k q", q=P
                    ),
                )

            def w_tile(vb, k):
                tiles, kc = w_sb[vb]
                return tiles[k // kc][:, k % kc]


            # xt loaded in groups of KG k-tiles (>=1 MiB per DMA)
            KG = 8
            NKG = NK // KG
            xt_r = xt_d[:, :].rearrange(
                "(g q p) (r c) -> p r g q c", p=P, q=KG, c=RB
            )

            def load_xt(r, issue=True):
                tiles = []
                for g in range(NKG):
                    t = xpool.tile(
                        [P, KG, RB], DT, name=f"xt_{r}_{g}", tag="xt",
                        bufs=2 * NKG,
                    )
                    if issue:
                        nc.sync.dma_start(out=t[:], in_=xt_r[:, r, g])
                    tiles.append(t)
                return tiles

            # First-block DMA issue order is tuned for time-to-first-matmul
            # and steady consumption: interleave vb0's W chunks with the xt
            # chunks they'll be consumed with; constants (needed only by the
            # first ACT/u-matmul, ~10us later) and the other W slabs follow.
            load_w(0, nsplit=8, issue=False)
            xt_tiles = load_xt(0, issue=False)
            issue_w(0, 0)
            nc.sync.dma_start(out=xt_tiles[0][:], in_=xt_r[:, 0, 0])
            issue_w(0, 1)
            issue_w(0, 2)
            issue_w(0, 3)
            nc.sync.dma_start(out=xt_tiles[1][:], in_=xt_r[:, 0, 1])
            issue_w(0, 4)
            issue_w(0, 5)
            issue_w(0, 6)
            issue_w(0, 7)
            nc.sync.dma_start(out=xt_tiles[2][:], in_=xt_r[:, 0, 2])
            nc.sync.dma_start(out=xt_tiles[3][:], in_=xt_r[:, 0, 3])

            # --- constants ---
            bct_sb = spool.tile([P, NVB], f32, name="bct_sb")
            nc.sync.dma_start(out=bct_sb[:], in_=bct_d[:, :])
            vwt_sb = spool.tile([P, NVB], f32, name="vwt_sb")
            nc.sync.dma_start(out=vwt_sb[:], in_=vwt_d[:, :])

            for vb in range(1, NVB):
                load_w(vb)

            # --- softmax state (exp runs per-rowblock, overlapped) ---
            # u lives on a single partition (engine PSUM/SBUF access must
            # start at partition 0); each batch's span is scaled in place
            # once its sum is known and DMA'd straight to its DRAM row.
            u2f = spool.tile([1, ROWS], f32, name="u2f")
            esum_all = spool.tile([1, NRB], f32, name="esum_all")
            etot = spool.tile([1, BC], f32, name="etot")
            rec1 = spool.tile([1, BC], f32, name="rec1")
            ones_sb = spool.tile([P, 1], f32r, name="ones_sb")
            nc.sync.dma_start(out=ones_sb[:], in_=ones_d[:, :])

            # --- PE warm-up -------------------------------------------------
            # The first real matmul can't start until ~8us of DMA lands, and
            # the PE takes ~3us of continuous work to leave its low DVFS
            # p-state.  Burn the wait on dummy matmuls over a memset tile so
            # the ramp happens off the critical path (results never read).
            warm_x = spool.tile([P, RB], bf16, name="warm_x")
            nc.vector.memset(warm_x[:], 0.0)
            warm_ps = pspool.tile([1, RB], f32, name="warm_ps")
            for _ in range(26):
                nc.tensor.matmul(
                    warm_ps[:], warm_x[:, 0:1], warm_x[:], start=True, stop=True
                )

            # The u-stage matmul for block (r, vb) is deferred until after
            # (r, vb+1)'s main matmuls are issued, so the PE never stalls
            # waiting for the ACT tanh to finish (ACT has a full vb-block
            # of matmul time to complete instead of being on the critical
            # path).  The drain of u_ps for rowblock r likewise lands
            # during rowblock r+1's first vb-block.
            def flush_u(pend):
                # u accumulation runs on the (otherwise idle) DVE:
                #   uacc[p, row] += vwt[p, vb] * tanh_vb[p, row]
                # with a single ones-matmul per rowblock doing the final
                # 128-partition reduction on the PE (1 instr instead of 8).
                r, vb, u_ps, tt, uacc = pend
                if vb == 0:
                    nc.vector.tensor_scalar_mul(
                        uacc[:], tt[:], vwt_sb[:, 0:1]
                    )
                else:
                    nc.vector.scalar_tensor_tensor(
                        uacc[:],
                        tt[:],
                        vwt_sb[:, vb : vb + 1],
                        uacc[:],
                        mybir.AluOpType.mult,
                        mybir.AluOpType.add,
                    )
                if vb == NVB - 1:
                    nc.tensor.matmul(
                        u_ps[:], ones_sb[:], uacc[:], start=True, stop=True
                    )
                    # No max subtraction: u is tanh-bounded so exp is safe.
                    nc.scalar.activation(
                        u2f[0:1, r * RB : (r + 1) * RB],
                        u_ps[:],
                        mybir.ActivationFunctionType.Exp,
                        accum_out=esum_all[0:1, r : r + 1],
                    )
                    if (r + 1) % RPB == 0:
                        # batch b complete: 1/sum, scale in place, store its
                        # DRAM row -- all overlapped with later rowblocks
                        # (the last batch is the only exposed tail).
                        b = r // RPB
                        nc.vector.tensor_reduce(
                            etot[0:1, b : b + 1],
                            esum_all[0:1, b * RPB : (b + 1) * RPB],
                            mybir.AxisListType.X,
                            mybir.AluOpType.add,
                        )
                        nc.vector.reciprocal(
                            rec1[0:1, b : b + 1], etot[0:1, b : b + 1]
                        )
                        # scale+store in halves so the first DMA overlaps the
                        # second multiply (matters for the last batch's tail)
                        HS = S // 2
                        for h in range(2):
                            lo = b * S + h * HS
                            nc.vector.tensor_scalar_mul(
                                u2f[0:1, lo : lo + HS],
                                u2f[0:1, lo : lo + HS],
                                rec1[0:1, b : b + 1],
                            )
                            nc.sync.dma_start(
                                out=att_d[b : b + 1, h * HS : (h + 1) * HS],
                                in_=u2f[0:1, lo : lo + HS],
                            )

            pend = None
            for r in range(NRB):
                u_ps = pspool.tile([1, RB], f32, name="u_ps", tag="ups", bufs=2)
                uacc = tpool.tile([P, RB], f32r, name="uacc", tag="uacc", bufs=2)
                for vb in range(NVB):
                    ps = pspool.tile([P, RB], f32, name="ps", tag="ps", bufs=2)
                    for k in range(NK):
                        nc.tensor.matmul(
                            ps[:],
                            w_tile(vb, k),
                            xt_tiles[k // KG][:, k % KG],
                            start=(k == 0),
                            stop=(k == NK - 1),
                        )
                    tt = tpool.tile([P, RB], f32, name="tt", tag="tt", bufs=3)
                    nc.scalar.activation(
                        tt[:],
                        ps[:],
                        mybir.ActivationFunctionType.Tanh,
                        bias=bct_sb[:, vb : vb + 1],
                        scale=1.0,
                    )
                    if pend is not None:
                        flush_u(pend)
                    pend = (r, vb, u_ps, tt, uacc)
                if r + 1 < NRB:
                    xt_tiles = load_xt(r + 1)
            flush_u(pend)

    _split_multiwait(nc)
    return nc


# ---------------------------------------------------------------------------
# Host entry point
# ---------------------------------------------------------------------------
def kernel(hidden, z, Ww, bw, Wz, bz, Vw, vb, w_a):
    _install_drain_patch()
    from concourse.bass_utils import run_bass_kernel_spmd

    np_main = ml_dtypes.bfloat16 if MAIN_DT == "bf16" else np.float32

    # ---- host-side shard prep ----
    hid_t = np.ascontiguousarray(
        np.asarray(hidden).astype(np_main).transpose(2, 1, 0)
    )  # [H2, B, S]
    z_t = np.ascontiguousarray(
        np.asarray(z).astype(np_main).transpose(2, 1, 0)
    )  # [H2, B, S]

    w_cat = np.concatenate(
        [np.asarray(Ww), np.asarray(Wz)], axis=0
    ).astype(np_main)  # [H, V]
    # reorder so tile (vb) is [P, NK*P] with per-partition-contiguous rows:
    # w_r[vb, p, k*P+q] = W[k*P+p, vb*P+q]
    w_r = np.ascontiguousarray(
        w_cat.reshape(NK, P, NVB, P).transpose(2, 1, 0, 3)
    ).reshape(NVB, P, NK * P)

    bias = (
        np.asarray(bw).astype(np.float64)
        + np.asarray(bz).astype(np.float64)
        + float(np.asarray(w_a)) * ALPHA_S
    ).astype(np.float32)  # [V]
    bct = np.ascontiguousarray(bias.reshape(NVB, P).T)  # [P, NVB]
    vwt = np.ascontiguousarray(
        np.asarray(Vw).astype(np.float32).reshape(NVB, P).T
    )  # [P, NVB]
    ones_col = np.ones((P, 1), dtype=np.float32)

    in_maps = []
    for c in range(NCORES):
        xt_c = np.empty((H, ROWS), dtype=np_main)
        xt_c[:H2] = hid_t[:, 2 * c : 2 * c + 2, :].reshape(H2, ROWS)
        xt_c[H2:] = z_t[:, 2 * c : 2 * c + 2, :].reshape(H2, ROWS)
        in_maps.append(
            {"xt": xt_c, "w": w_r, "bct": bct, "vwt": vwt, "ones": ones_col}
        )

    nc = _build_nc()
    res = run_bass_kernel_spmd(nc, in_maps, list(range(NCORES)))

    out = np.empty((S, B, 1), dtype=np.float32)
    for c in range(NCORES):
        att = res.results[c]["att"]  # [BC, S]
        for b in range(BC):
            out[:, 2 * c + b, 0] = att[b]
    return out



# revision 23
# speedup vs baseline: 1.0045x; 1.0045x over previous
"""Trainium2 Bass kernel for nn_DualAttention (S=2048, B=16, H2=2048, V=1024).

Computation (per the reference):
    sum_w = hidden @ Ww + bw + z @ Wz + bz + w_a*0.5        [S, B, V]
    u     = tanh(sum_w) @ Vw + vb                            [S, B, 1]
    out   = softmax(u, axis=0)                               [S, B, 1]

Strategy
--------
Data-parallel over batch: 16 batches -> 2 per NeuronCore (8 cores).
Host-side prep per core:
  * concat hidden/z along the hidden axis -> X [ROWS=4096, H=4096]
    (rows are b-major: row = b_local*2048 + s)
  * transpose to xt = X^T [H, ROWS], cast to the matmul dtype
  * W = concat([Ww, Wz], 0) [H, V], reordered into per-(vb,k) 128x128
    tiles; bias = bw + bz + 0.5*w_a
Device kernel (per core), W-stationary matmul with psum layout [v, rows]:
  for each rowblock (RB rows):
    load xt[:, rowblock] into SBUF (one [128, RB] tile per k)
    for vb in 0..7:                       # 128-wide slices of V
      psum[vb] += sum_k W[vb,k].T @ xt[k]      (32 accumulating matmuls)
      t = tanh(psum + bias_vb)            # one ACT op, per-partition bias
      u_psum += Vw[vb].T @ t              # [1, RB] second-stage matmul (f32r)
    u_scratch[rowblock] = u_psum          # via SBUF bounce -> DRAM
  softmax over s per batch (no max subtraction: u is tanh-bounded):
    DMA u_scratch -> [2, 2048], exp+rowsum on ACT (in place),
    reciprocal + scale on DVE (in place), DMA out [2, 2048].

The vb scalar is dropped: softmax is shift-invariant.

MAIN_DT selects the matmul dtype: "bf16" (faster, ~1e-2 rel err) or
"f32r" (fp32 data with the PE's fast rounded-fp32 mode, ~1e-3 rel err).
"""

import numpy as np
import ml_dtypes

# ---------------------------------------------------------------------------
# Problem constants (hardcoded; kernel.py must be self-contained)
# ---------------------------------------------------------------------------
S, B, H2, V = 2048, 16, 2048, 1024
ALPHA_S = 0.5
NCORES = 8
BC = B // NCORES            # local batches per core
ROWS = S * BC               # 4096 rows per core (b-major)
H = 2 * H2                  # 4096 contraction dim (hidden ++ z)
P = 128
NK = H // P                 # 32
NVB = V // P                # 8

MAIN_DT = "bf16"            # "bf16" | "f32r"
RB = 512 if MAIN_DT == "bf16" else 256
NRB = ROWS // RB


# ---------------------------------------------------------------------------
# Workarounds for this walrus build's 1-sync-wait-per-instruction limit
# ---------------------------------------------------------------------------
def _install_drain_patch():
    import concourse.mybir as mybir
    from concourse.tile import TileContext
    from concourse.vector_clock import ScopedClock

    def _drain_and_barrier(self, tick_clock, wait_clock):
        nc = self.nc
        drain_inst = nc.sync.drain()
        wait_clock.add_sem_waits(
            drain_inst.ins, ScopedClock({None: tick_clock.global_clock})
        )
        si = drain_inst.ins.sync_info
        if si is not None:
            waits = list(si.on_wait)
            if len(waits) > 1:
                si.on_wait = [waits[0]]
                for w in waits[1:]:
                    nop = nc.sync.nop(nofuse=True)
                    nop.ins.sync_info = mybir.SyncInfo(on_wait=[w], on_update=[])
        nc.all_engine_barrier()
        assert self.sems is not None
        popped = nc._tile_sem_poison_stack.pop()
        assert popped is self._sem_poison
        nc.clear_and_free_semaphores(list(self.sems.allocated().values()))
        nc.all_engine_barrier()

    TileContext._drain_and_barrier = _drain_and_barrier


def _split_multiwait(nc):
    """Hoist extra sync waits onto same-engine event-semaphore instructions
    inserted just before the carrying instruction."""
    import concourse.mybir as mybir

    counter = 0
    for fn in nc.m.functions:
        for bb in fn.blocks:
            insts = bb.instructions
            new_list = []
            changed = False
            for inst in insts:
                si = inst.sync_info
                if si is not None:
                    waits = list(si.on_wait)
                    if len(waits) > 1:
                        for w in waits[:-1]:
                            counter += 1
                            nop = mybir.InstEventSemaphore(
                                name=f"I-mwsplit-{counter}"
                            )
                            nop.engine = inst.engine
                            nop.bass_nofuse = True
                            nop.sync_info = mybir.SyncInfo(
                                on_wait=[w], on_update=[]
                            )
                            nc.register_instruction(nop)
                            new_list.append(nop)
                        si.on_wait = [waits[-1]]
                        changed = True
                new_list.append(inst)
            if changed:
                bb.instructions = new_list
    return counter


# ---------------------------------------------------------------------------
# Kernel build
# ---------------------------------------------------------------------------
def _build_nc():
    import concourse.bass as bass
    import concourse.mybir as mybir
    from concourse.tile import TileContext

    f32 = mybir.dt.float32
    f32r = mybir.dt.float32r
    bf16 = mybir.dt.bfloat16
    DT = mybir.dt.bfloat16 if MAIN_DT == "bf16" else f32r

    nc = bass.Bass()
    # W pre-tiled host-side: tile (vb, k) is [P, 128] contiguous
    w_d = nc.declare_dram_parameter("w", [NVB, P, NK * P], DT, isOutput=False)
    xt_d = nc.declare_dram_parameter("xt", [H, ROWS], DT, isOutput=False)
    bct_d = nc.declare_dram_parameter("bct", [P, NVB], f32, isOutput=False)
    vwt_d = nc.declare_dram_parameter("vwt", [P, NVB], f32, isOutput=False)
    ones_d = nc.declare_dram_parameter("ones", [P, 1], f32r, isOutput=False)
    att_d = nc.declare_dram_parameter("att", [BC, S], f32, isOutput=True)

    RPB = NRB // BC             # rowblocks per local batch

    with TileContext(nc) as tc:
        with (
            tc.tile_pool(name="wpool", bufs=1) as wpool,
            tc.tile_pool(name="xpool", bufs=1) as xpool,
            tc.tile_pool(name="tpool", bufs=1) as tpool,
            tc.tile_pool(name="spool", bufs=1) as spool,
            tc.tile_pool(name="pspool", bufs=1, space="PSUM") as pspool,
        ):
            # --- resident weights: vb0's tiles first (fast start), then rest
            # each vb's weights may be split into `nsplit` tiles along k so
            # the first matmuls can start before the whole slab lands
            w_sb = [None] * NVB

            def load_w(vb, nsplit=1, issue=True):
                kc = NK // nsplit
                tiles = []
                for j in range(nsplit):
                    t = wpool.tile([P, kc, P], DT, name=f"w_{vb}_{j}")
                    tiles.append(t)
                w_sb[vb] = (tiles, kc)
                if issue:
                    for j in range(nsplit):
                        issue_w(vb, j)

            def issue_w(vb, j):
                tiles, kc = w_sb[vb]
                nc.sync.dma_start(
                    out=tiles[j][:],
                    in_=w_d[vb, :, j * kc * P : (j + 1) * kc * P].rearrange(
                        "p (k q) -> p k q", q=P
                    ),
                )

            def w_tile(vb, k):
                tiles, kc = w_sb[vb]
                return tiles[k // kc][:, k % kc]


            # xt loaded in groups of KG k-tiles (>=1 MiB per DMA)
            KG = 8
            NKG = NK // KG
            xt_r = xt_d[:, :].rearrange(
                "(g q p) (r c) -> p r g q c", p=P, q=KG, c=RB
            )

            def load_xt(r, issue=True):
                tiles = []
                for g in range(NKG):
                    t = xpool.tile(
                        [P, KG, RB], DT, name=f"xt_{r}_{g}", tag="xt",
                        bufs=2 * NKG,
                    )
                    if issue:
                        nc.sync.dma_start(out=t[:], in_=xt_r[:, r, g])
                    tiles.append(t)
                return tiles

            # First-block DMA issue order is tuned for time-to-first-matmul
            # and steady consumption: interleave vb0's W chunks with the xt
            # chunks they'll be consumed with; constants (needed only by the
            # first ACT/u-matmul, ~10us later) and the other W slabs follow.
            load_w(0, nsplit=8, issue=False)
            xt_tiles = load_xt(0, issue=False)
            issue_w(0, 0)
            nc.sync.dma_start(out=xt_tiles[0][:], in_=xt_r[:, 0, 0])
            issue_w(0, 1)
            issue_w(0, 2)
            issue_w(0, 3)
            nc.sync.dma_start(out=xt_tiles[1][:], in_=xt_r[:, 0, 1])
            issue_w(0, 4)
            issue_w(0, 5)
            issue_w(0, 6)
            issue_w(0, 7)
            nc.sync.dma_start(out=xt_tiles[2][:], in_=xt_r[:, 0, 2])
            nc.sync.dma_start(out=xt_tiles[3][:], in_=xt_r[:, 0, 3])

            # --- constants ---
            bct_sb = spool.tile([P, NVB], f32, name="bct_sb")
            nc.sync.dma_start(out=bct_sb[:], in_=bct_d[:, :])
            vwt_sb = spool.tile([P, NVB], f32, name="vwt_sb")
            nc.sync.dma_start(out=vwt_sb[:], in_=vwt_d[:, :])

            for vb in range(1, NVB):
                load_w(vb)

            # --- softmax state (exp runs per-rowblock, overlapped) ---
            # u lives on a single partition (engine PSUM/SBUF access must
            # start at partition 0); each batch's span is scaled in place
            # once its sum is known and DMA'd straight to its DRAM row.
            u2f = spool.tile([1, ROWS], f32, name="u2f")
            esum_all = spool.tile([1, NRB], f32, name="esum_all")
            etot = spool.tile([1, BC], f32, name="etot")
            rec1 = spool.tile([1, BC], f32, name="rec1")
            ones_sb = spool.tile([P, 1], f32r, name="ones_sb")
            nc.sync.dma_start(out=ones_sb[:], in_=ones_d[:, :])

            # --- PE warm-up -------------------------------------------------
            # The first real matmul can't start until ~8us of DMA lands, and
            # the PE takes ~3us of continuous work to leave its low DVFS
            # p-state.  Burn the wait on dummy matmuls over a memset tile so
            # the ramp happens off the critical path (results never read).
            # Sized to end just as the first xt/W chunks land (~14us): the
            # ramp runs at ~427ns/matmul, and any idle gap after the dummies
            # would drop the p-state right back down.
            warm_x = spool.tile([P, RB], bf16, name="warm_x")
            nc.vector.memset(warm_x[:], 0.0)
            warm_ps = pspool.tile([1, RB], f32, name="warm_ps")
            for _ in range(13):
                nc.tensor.matmul(
                    warm_ps[:], warm_x[:, 0:1], warm_x[:], start=True, stop=True
                )

            # The u-stage matmul for block (r, vb) is deferred until after
            # (r, vb+1)'s main matmuls are issued, so the PE never stalls
            # waiting for the ACT tanh to finish (ACT has a full vb-block
            # of matmul time to complete instead of being on the critical
            # path).  The drain of u_ps for rowblock r likewise lands
            # during rowblock r+1's first vb-block.
            def flush_u(pend):
                # u accumulation runs on the (otherwise idle) DVE:
                #   uacc[p, row] += vwt[p, vb] * tanh_vb[p, row]
                # with a single ones-matmul per rowblock doing the final
                # 128-partition reduction on the PE (1 instr instead of 8).
                r, vb, u_ps, tt, uacc = pend
                if vb == 0:
                    nc.vector.tensor_scalar_mul(
                        uacc[:], tt[:], vwt_sb[:, 0:1]
                    )
                else:
                    nc.vector.scalar_tensor_tensor(
                        uacc[:],
                        tt[:],
                        vwt_sb[:, vb : vb + 1],
                        uacc[:],
                        mybir.AluOpType.mult,
                        mybir.AluOpType.add,
                    )
                if vb == NVB - 1:
                    nc.tensor.matmul(
                        u_ps[:], ones_sb[:], uacc[:], start=True, stop=True
                    )
                    # No max subtraction: u is tanh-bounded so exp is safe.
                    nc.scalar.activation(
                        u2f[0:1, r * RB : (r + 1) * RB],
                        u_ps[:],
                        mybir.ActivationFunctionType.Exp,
                        accum_out=esum_all[0:1, r : r + 1],
                    )
                    if (r + 1) % RPB == 0:
                        # batch b complete: 1/sum, scale in place, store its
                        # DRAM row -- all overlapped with later rowblocks
                        # (the last batch is the only exposed tail).
                        b = r // RPB
                        nc.vector.tensor_reduce(
                            etot[0:1, b : b + 1],
                            esum_all[0:1, b * RPB : (b + 1) * RPB],
                            mybir.AxisListType.X,
                            mybir.AluOpType.add,
                        )
                        nc.vector.reciprocal(
                            rec1[0:1, b : b + 1], etot[0:1, b : b + 1]
                        )
                        # scale+store in halves so the first DMA overlaps the
                        # second multiply (matters for the last batch's tail)
                        HS = S // 2
                        for h in range(2):
                            lo = b * S + h * HS
                            nc.vector.tensor_scalar_mul(
                                u2f[0:1, lo : lo + HS],
                                u2f[0:1, lo : lo + HS],
                                rec1[0:1, b : b + 1],
                            )
                            nc.sync.dma_start(
                                out=att_d[b : b + 1, h * HS : (h + 1) * HS],
                                in_=u2f[0:1, lo : lo + HS],
                            )

            pend = None
            for r in range(NRB):
                u_ps = pspool.tile([1, RB], f32, name="u_ps", tag="ups", bufs=2)
                uacc = tpool.tile([P, RB], f32r, name="uacc", tag="uacc", bufs=2)
                for vb in range(NVB):
                    ps = pspool.tile([P, RB], f32, name="ps", tag="ps", bufs=2)
                    for k in range(NK):
                        nc.tensor.matmul(
                            ps[:],
                            w_tile(vb, k),
                            xt_tiles[k // KG][:, k % KG],
                            start=(k == 0),
                            stop=(k == NK - 1),
                        )
                    tt = tpool.tile([P, RB], f32, name="tt", tag="tt", bufs=3)
                    nc.scalar.activation(
                        tt[:],
                        ps[:],
                        mybir.ActivationFunctionType.Tanh,
                        bias=bct_sb[:, vb : vb + 1],
                        scale=1.0,
                    )
                    if pend is not None:
                        flush_u(pend)
                    pend = (r, vb, u_ps, tt, uacc)
                if r + 1 < NRB:
                    xt_tiles = load_xt(r + 1)
            flush_u(pend)

    _split_multiwait(nc)
    return nc


# ---------------------------------------------------------------------------
# Host entry point
# ---------------------------------------------------------------------------
def kernel(hidden, z, Ww, bw, Wz, bz, Vw, vb, w_a):
    _install_drain_patch()
    from concourse.bass_utils import run_bass_kernel_spmd

    np_main = ml_dtypes.bfloat16 if MAIN_DT == "bf16" else np.float32

    # ---- host-side shard prep ----
    hid_t = np.ascontiguousarray(
        np.asarray(hidden).astype(np_main).transpose(2, 1, 0)
    )  # [H2, B, S]
    z_t = np.ascontiguousarray(
        np.asarray(z).astype(np_main).transpose(2, 1, 0)
    )  # [H2, B, S]

    w_cat = np.concatenate(
        [np.asarray(Ww), np.asarray(Wz)], axis=0
    ).astype(np_main)  # [H, V]
    # reorder so tile (vb) is [P, NK*P] with per-partition-contiguous rows:
    # w_r[vb, p, k*P+q] = W[k*P+p, vb*P+q]
    w_r = np.ascontiguousarray(
        w_cat.reshape(NK, P, NVB, P).transpose(2, 1, 0, 3)
    ).reshape(NVB, P, NK * P)

    bias = (
        np.asarray(bw).astype(np.float64)
        + np.asarray(bz).astype(np.float64)
        + float(np.asarray(w_a)) * ALPHA_S
    ).astype(np.float32)  # [V]
    bct = np.ascontiguousarray(bias.reshape(NVB, P).T)  # [P, NVB]
    vwt = np.ascontiguousarray(
        np.asarray(Vw).astype(np.float32).reshape(NVB, P).T
    )  # [P, NVB]
    ones_col = np.ones((P, 1), dtype=np.float32)

    in_maps = []
    for c in range(NCORES):
        xt_c = np.empty((H, ROWS), dtype=np_main)
        xt_c[:H2] = hid_t[:, 2 * c : 2 * c + 2, :].reshape(H2, ROWS)
        xt_c[H2:] = z_t[:, 2 * c : 2 * c + 2, :].reshape(H2, ROWS)
        in_maps.append(
            {"xt": xt_c, "w": w_r, "bct": bct, "vwt": vwt, "ones": ones_col}
        )

    nc = _build_nc()
    res = run_bass_kernel_spmd(nc, in_maps, list(range(NCORES)))

    out = np.empty((S, B, 1), dtype=np.float32)
    for c in range(NCORES):
        att = res.results[c]["att"]  # [BC, S]
        for b in range(BC):
            out[:, 2 * c + b, 0] = att[b]
    return out

